# revision 3
# baseline (speedup 1.0000x reference)
"""BCQLinear packed forward on 8 Trainium2 NeuronCores.

Column-parallel (tensor-parallel) sharding: binary/alpha/bias are sharded
along out_features (dim 0, 4096 -> 8 x 512); the input activations are
replicated. Each core dequantizes its weight shard
    W[o, i] = sum_b alpha[o, g, b] * binary[o, g, a, b],   i = 128 g + a
on the vector engine in bf16, transposes each 128x128 group block through
the PE array (one bf16 transpose per group), and runs W-stationary bf16
matmuls  out[o, t] = Wt[:, o]^T @ x[:, t]  with 512-token moving blocks.
o-tiles are pipelined: dequant of o-tile N+1 (DVE) overlaps the matmuls
of o-tile N (PE). The host concatenates the 8 output shards along o.

Shapes are hardcoded for the problem instance:
  input  [2, 1024, 4096] f32
  binary [4096, 32, 128, 3] f32 (+-1)
  alpha  [4096, 32, 3] f32
  bias   [4096] f32
"""

import numpy as np
from contextlib import ExitStack

import bass_rust
import concourse.bass as bass
import concourse.mybir as mybir
import concourse.tile as tile
from concourse.bass_utils import run_bass_kernel_spmd
from concourse.masks import make_identity


def _legalize_waits(nc, max_waits=1):
    """Walrus codegen allows only one sync-wait on (at least) DVE
    TensorTensor instructions. Move excess waits onto injected same-engine
    NoOps placed immediately before the instruction (program order per
    engine preserves the semantics)."""
    seq = 0
    for fn in nc.m.functions:
        for blk in fn.blocks:
            new_insts = []
            changed = False
            for inst in blk.instructions:
                si = inst.sync_info
                if si is not None and len(si.on_wait) > max_waits:
                    waits = list(si.on_wait)
                    for w in waits[:-max_waits]:
                        nop = mybir.InstNoOp(name=f"wlegal-{seq}")
                        seq += 1
                        nop.engine = inst.engine
                        nop.sync_info = bass_rust.SyncInfo(
                            on_wait=[w], on_update=[])
                        new_insts.append(nop)
                    inst.sync_info = bass_rust.SyncInfo(
                        on_wait=waits[-max_waits:],
                        on_update=list(si.on_update))
                    changed = True
                new_insts.append(inst)
            if changed:
                blk.instructions = new_insts

P = 128          # partitions
N_CORES = 8
B, S = 2, 1024
MS = B * S       # 2048 tokens
I = 4096         # in_features
O = 4096         # out_features
O_SH = O // N_CORES  # 512 per core
G, A, NB = 32, 128, 3
KT = I // P      # 32 contraction tiles (== G since A == P)
OT = O_SH // P   # 4 o-tiles per core
TB = 4           # token blocks
TS = MS // TB    # 512 tokens per block

F32 = mybir.dt.float32
BF16 = mybir.dt.bfloat16
FP8 = mybir.dt.float8e4

_CACHED = {}


def build_nc(gh: int = 8, x_split: int = 1) -> bass.Bass:
    nc = bass.Bass("TRN2", target_bir_lowering=False, debug=False)

    # Host-staged layouts (pure relayouts / dtype casts of sharded inputs):
    #  xt    [TB, P, KT, TS] bf16 : xt[tb,p,k,t] = x[tb*TS+t, k*128+p]
    #  bperm [O_SH, NB, G, A] fp8 : binary shard, bit axis outward (+-1 exact)
    #  alpha [O_SH, G, NB] f32
    #  biassh[OT, P] f32          : bias shard split into o-tiles
    xt_d = nc.dram_tensor("xt", [TB, P, KT, TS], BF16, kind="ExternalInput").ap()
    b_d = nc.dram_tensor("bperm", [O_SH, NB, G, A], FP8, kind="ExternalInput").ap()
    al_d = nc.dram_tensor("alpha", [O_SH, G, NB], F32, kind="ExternalInput").ap()
    bias_d = nc.dram_tensor("biassh", [OT, P], F32, kind="ExternalInput").ap()
    out_d = nc.dram_tensor("out", [OT, P, MS], F32, kind="ExternalOutput").ap()

    mult = mybir.AluOpType.mult
    add = mybir.AluOpType.add

    with tile.TileContext(nc) as tc, ExitStack() as ctx:
        const = ctx.enter_context(tc.tile_pool(name="const", bufs=1))
        xpool = ctx.enter_context(tc.tile_pool(name="x", bufs=TB))
        wtpool = ctx.enter_context(tc.tile_pool(name="wt", bufs=2))
        bpool = ctx.enter_context(tc.tile_pool(name="bin", bufs=6))
        wpool = ctx.enter_context(tc.tile_pool(name="w", bufs=2))
        opool = ctx.enter_context(tc.tile_pool(name="o", bufs=3))
        ps_mm = ctx.enter_context(tc.tile_pool(name="psmm", bufs=2, space="PSUM"))
        ps_tr = ctx.enter_context(tc.tile_pool(name="pstr", bufs=4, space="PSUM"))

        identb = const.tile([P, P], BF16)
        make_identity(nc, identb)
        al_sb = const.tile([P, OT, G * NB], F32)
        nc.sync.dma_start(al_sb, al_d.rearrange("(ot p) g nb -> p ot (g nb)", p=P))
        bias_sb = const.tile([P, OT], F32)
        nc.sync.dma_start(bias_sb, bias_d.rearrange("ot p -> p ot"))
        al3 = al_sb.rearrange("p ot (g nb) -> p ot g nb", nb=NB)

        # Input DMAs interleaved so dequant of o-tile ot and matmuls of
        # token-block tb are fed in the order the pipeline consumes them.
        x_tiles = [xpool.tile([P, KT, TS], BF16, tag="x", name=f"x{tb}")
                   for tb in range(TB)]
        b_tiles = {}

        def emit_b_dma(ot):
            tiles = []
            for b in range(NB):
                b_sb = bpool.tile([P, G, A], FP8)
                nc.sync.dma_start(b_sb, b_d[ot * P:(ot + 1) * P, b])
                tiles.append(b_sb)
            b_tiles[ot] = tiles

        def emit_x_dma(tb):
            if x_split == 1:
                nc.sync.dma_start(x_tiles[tb], xt_d[tb])
            else:
                ksz = KT // x_split
                for s in range(x_split):
                    ksl = slice(s * ksz, (s + 1) * ksz)
                    nc.sync.dma_start(x_tiles[tb][:, ksl], xt_d[tb, :, ksl])

        emit_b_dma(0)
        emit_x_dma(0)
        emit_b_dma(1)
        emit_x_dma(1)
        emit_b_dma(2)
        emit_x_dma(2)
        emit_b_dma(3)
        emit_x_dma(3)

        def emit_dequant(ot):
            # W[o_p, g, a] = sum_b alpha[o_p, g, b] * binary[o_p, g, a, b]
            # accumulated in bf16 on DVE, then one bf16 PE transpose per
            # group into Wt[a_p, g, o] (SBUF via ACT copy).
            bt = b_tiles[ot]
            wt_sb = wtpool.tile([P, KT, P], BF16, tag="wt")
            w_sb = wpool.tile([P, G, A], BF16, tag="w")
            t_sb = wpool.tile([P, G, A], BF16, tag="t")
            for gh0 in range(0, G, gh):
                gsl = slice(gh0, gh0 + gh)

                def al_bc(b):
                    return al3[:, ot, gsl, b:b + 1].to_broadcast([P, gh, A])

                nc.vector.tensor_tensor(w_sb[:, gsl], bt[0][:, gsl], al_bc(0), mult)
                nc.vector.tensor_tensor(t_sb[:, gsl], bt[1][:, gsl], al_bc(1), mult)
                nc.vector.tensor_tensor(w_sb[:, gsl], w_sb[:, gsl], t_sb[:, gsl], add)
                nc.vector.tensor_tensor(t_sb[:, gsl], bt[2][:, gsl], al_bc(2), mult)
                nc.vector.tensor_tensor(w_sb[:, gsl], w_sb[:, gsl], t_sb[:, gsl], add)
                for g in range(gh0, gh0 + gh):
                    ps = ps_tr.tile([P, P], BF16)
                    nc.tensor.matmul(ps, w_sb[:, g], identb, is_transpose=True,
                                     start=True, stop=True)
                    nc.scalar.copy(wt_sb[:, g, :], ps)
            return wt_sb

        def emit_mm(wt_sb, ot, tb):
            # out[o_p, t] = sum_k Wt[:, k, o]^T @ x[:, k, t]  (bf16, W-stationary)
            ps = ps_mm.tile([P, TS], F32)
            for k in range(KT):
                nc.tensor.matmul(ps, wt_sb[:, k, :], x_tiles[tb][:, k, :],
                                 start=(k == 0), stop=(k == KT - 1))
            out_sb = opool.tile([P, TS], F32)
            nc.vector.tensor_tensor(
                out_sb, ps, bias_sb[:, ot:ot + 1].to_broadcast([P, TS]), add)
            nc.gpsimd.dma_start(out_d[ot, :, tb * TS:(tb + 1) * TS], out_sb)

        for ot in range(OT):
            wt_sb = emit_dequant(ot)
            for tb in range(TB):
                emit_mm(wt_sb, ot, tb)

    _legalize_waits(nc)
    return nc


def _stage_inputs(input, binary, alpha, bias):
    bf16 = mybir.dt.np(BF16)
    fp8 = mybir.dt.np(FP8)
    x = np.ascontiguousarray(np.asarray(input, dtype=np.float32)).reshape(MS, I)
    # xt[tb, p, k, t] = x[tb*TS + t, k*128 + p]
    xt = np.ascontiguousarray(
        x.reshape(TB, TS, KT, P).transpose(0, 3, 2, 1)).astype(bf16)
    # binary is strictly +-1, exactly representable in fp8e4 — lossless cast.
    bperm = np.ascontiguousarray(
        np.asarray(binary, dtype=np.float32).transpose(0, 3, 1, 2)).astype(fp8)
    alpha = np.ascontiguousarray(np.asarray(alpha, dtype=np.float32))
    bias = np.asarray(bias, dtype=np.float32)

    in_maps = []
    for c in range(N_CORES):
        sl = slice(c * O_SH, (c + 1) * O_SH)
        in_maps.append({
            "xt": xt,
            "bperm": np.ascontiguousarray(bperm[sl]),
            "alpha": np.ascontiguousarray(alpha[sl]),
            "biassh": np.ascontiguousarray(bias[sl].reshape(OT, P)),
        })
    return in_maps


def kernel(input, binary, alpha, bias, _trace=False, _gh=8, _x_split=1,
           **_legacy):
    key = (_gh, _x_split)
    if key not in _CACHED:
        _CACHED[key] = build_nc(gh=_gh, x_split=_x_split)
    nc = _CACHED[key]
    in_maps = _stage_inputs(input, binary, alpha, bias)
    res = run_bass_kernel_spmd(nc, in_maps, core_ids=list(range(N_CORES)),
                               trace=_trace)
    # out core shard [OT, P, MS] -> [MS, O_SH]
    shards = [res.results[c]["out"].transpose(2, 0, 1).reshape(MS, O_SH)
              for c in range(N_CORES)]
    out = np.concatenate(shards, axis=1).reshape(B, S, O).astype(np.float32)
    kernel.last_result = res
    return out


# revision 10
# speedup vs baseline: 1.1722x; 1.1722x over previous
"""BCQLinear packed forward on 8 Trainium2 NeuronCores.

Column-parallel (tensor-parallel) sharding: binary/alpha/bias are sharded
along out_features (dim 0, 4096 -> 8 x 512); the input activations are
replicated. Each core dequantizes its weight shard
    W[o, i] = sum_b alpha[o, g, b] * binary[o, g, a, b],   i = 128 g + a
on the vector engine (gpsimd-assisted) in bf16, transposes each 128x128
group block through the PE array (one bf16 transpose per group), and runs
W-stationary bf16 matmuls  out[o, t] = Wt[:, o]^T @ x[:, t]  over token
blocks. Cells (o-tile, token-block) are emitted in DMA/dequant arrival
order so the PE never starves during the fill phase; bias is added on the
Activation engine during the PSUM->SBUF copy (per-partition bias, since
out is [o_p, t]). The host concatenates the 8 output shards along o.

Shapes are hardcoded for the problem instance:
  input  [2, 1024, 4096] f32
  binary [4096, 32, 128, 3] f32 (+-1)
  alpha  [4096, 32, 3] f32
  bias   [4096] f32
"""

import numpy as np
from contextlib import ExitStack

import bass_rust
import concourse.bass as bass
import concourse.mybir as mybir
import concourse.tile as tile
from concourse.bass_utils import run_bass_kernel_spmd
from concourse.masks import make_identity


def _legalize_waits(nc, max_waits=1):
    """Walrus codegen allows only one sync-wait on (at least) DVE
    TensorTensor instructions. Move excess waits onto injected same-engine
    NoOps placed immediately before the instruction (program order per
    engine preserves the semantics)."""
    seq = 0
    for fn in nc.m.functions:
        for blk in fn.blocks:
            new_insts = []
            changed = False
            for inst in blk.instructions:
                si = inst.sync_info
                if si is not None and len(si.on_wait) > max_waits:
                    waits = list(si.on_wait)
                    for w in waits[:-max_waits]:
                        nop = mybir.InstNoOp(name=f"wlegal-{seq}")
                        seq += 1
                        nop.engine = inst.engine
                        nop.sync_info = bass_rust.SyncInfo(
                            on_wait=[w], on_update=[])
                        new_insts.append(nop)
                    inst.sync_info = bass_rust.SyncInfo(
                        on_wait=waits[-max_waits:],
                        on_update=list(si.on_update))
                    changed = True
                new_insts.append(inst)
            if changed:
                blk.instructions = new_insts

P = 128          # partitions
N_CORES = 8
B, S = 2, 1024
MS = B * S       # 2048 tokens
I = 4096         # in_features
O = 4096         # out_features
O_SH = O // N_CORES  # 512 per core
G, A, NB = 32, 128, 3
KT = I // P      # 32 contraction tiles (== G since A == P)
OT = O_SH // P   # 4 o-tiles per core

F32 = mybir.dt.float32
BF16 = mybir.dt.bfloat16
FP8 = mybir.dt.float8e4

_CACHED = {}


def build_nc(gh: int = 8, x_split: int = 2, tb: int = 8, pool_assist: int = 1,
             dummies: int = 30, out_bf16: int = 1, zigzag: int = 1) -> bass.Bass:
    TB = tb
    TS = MS // TB
    ODT = BF16 if out_bf16 else F32

    nc = bass.Bass("TRN2", target_bir_lowering=False, debug=False)

    # Host-staged layouts (pure relayouts / dtype casts of sharded inputs):
    #  xt    [TB, P, KT, TS] bf16 : xt[tb,p,k,t] = x[tb*TS+t, k*128+p]
    #  bperm [O_SH, NB, G, A] fp8 : binary shard, bit axis outward (+-1 exact)
    #  alpha [O_SH, G, NB] f32
    #  biassh[OT, P] f32          : bias shard split into o-tiles
    xt_d = nc.dram_tensor("xt", [TB, P, KT, TS], BF16, kind="ExternalInput").ap()
    b_d = nc.dram_tensor("bperm", [O_SH, NB, G, A], FP8, kind="ExternalInput").ap()
    al_d = nc.dram_tensor("alpha", [O_SH, G, NB], F32, kind="ExternalInput").ap()
    bias_d = nc.dram_tensor("biassh", [OT, P], F32, kind="ExternalInput").ap()
    out_d = nc.dram_tensor("out", [OT, P, MS], ODT, kind="ExternalOutput").ap()

    mult = mybir.AluOpType.mult
    add = mybir.AluOpType.add

    with tile.TileContext(nc) as tc, ExitStack() as ctx:
        const = ctx.enter_context(tc.tile_pool(name="const", bufs=1))
        xpool = ctx.enter_context(tc.tile_pool(name="x", bufs=TB))
        wtpool = ctx.enter_context(tc.tile_pool(name="wt", bufs=4))
        bpool = ctx.enter_context(tc.tile_pool(name="bin", bufs=5))
        wpool = ctx.enter_context(tc.tile_pool(name="w", bufs=2))
        opool = ctx.enter_context(tc.tile_pool(name="o", bufs=4))
        ps_mm = ctx.enter_context(tc.tile_pool(name="psmm", bufs=3, space="PSUM"))
        ps_tr = ctx.enter_context(tc.tile_pool(name="pstr", bufs=4, space="PSUM"))

        identb = const.tile([P, P], BF16)
        make_identity(nc, identb)
        al_sb = const.tile([P, OT, G * NB], F32)
        bias_sb = const.tile([P, OT], F32)
        al3 = al_sb.rearrange("p ot (g nb) -> p ot g nb", nb=NB)

        # PE p-state warm-up: junk transposes with no data deps keep the PE
        # busy from t~1us so the ramp (low/mid clock for the first 3us of
        # continuous activity) is spent before real matmuls arrive.
        if dummies:
            ps_junk = ps_tr.tile([P, P], BF16, tag="junk", bufs=1)
            for _ in range(dummies):
                nc.tensor.matmul(ps_junk, identb, identb, is_transpose=True,
                                 start=True, stop=True)

        x_tiles = [xpool.tile([P, KT, TS], BF16, tag="x", name=f"x{t}")
                   for t in range(TB)]
        b_tiles = {}

        def emit_b_dma(ot):
            tiles = []
            for b in range(NB):
                b_sb = bpool.tile([P, G, A], FP8)
                nc.sync.dma_start(b_sb, b_d[ot * P:(ot + 1) * P, b])
                tiles.append(b_sb)
            b_tiles[ot] = tiles

        def emit_x_dma(t):
            ksz = KT // x_split
            for s in range(x_split):
                ksl = slice(s * ksz, (s + 1) * ksz)
                nc.sync.dma_start(x_tiles[t][:, ksl], xt_d[t, :, ksl])

        # --- Input DMA order + availability model (serial DMA resource at
        # ~360 GB/s). Interleave b(ot) and x(t) in consumption order.
        BW = 1.0 / (16 * 22.4)   # us per byte (dma_bus_bytes_per_ns*16)
        b_bytes = P * G * A      # fp8, per bit-plane
        x_bytes = P * KT * TS * 2
        x_av = [0.0] * TB
        d_dma = [0.0] * OT
        dma_t = 0.5              # trigger pipeline latency
        nc.sync.dma_start(al_sb, al_d.rearrange("(ot p) g nb -> p ot (g nb)", p=P))
        nc.sync.dma_start(bias_sb, bias_d.rearrange("ot p -> p ot"))
        dma_t += (OT * G * NB * 4 + OT * 4) * P * BW / 1000

        # Front-loaded x: binary planes only need to land ~one dequant
        # ahead of their PE phase, so they yield early DMA slots to x.
        seq = ["b0", "x0", "x1", "x2", "b1", "x3", "x4", "x5", "b2",
               "x6", "x7", "b3"] if TB == 8 else (
              ["b0", "x0", "b1", "x1", "b2", "x2", "b3", "x3"])
        for item in seq:
            idx = int(item[1:])
            if item[0] == "b":
                emit_b_dma(idx)
                dma_t += 3 * b_bytes * BW / 1000
                d_dma[idx] = dma_t
            else:
                emit_x_dma(idx)
                dma_t += x_bytes * BW / 1000
                x_av[idx] = dma_t

        # dequant availability: DVE chain, 13.6us/ot with pool assist
        deq_dt = 13.6 if pool_assist else 18.3
        d_av = []
        t_dve = 1.0
        for ot in range(OT):
            t_dve = max(t_dve, d_dma[ot] + 0.9) + deq_dt
            d_av.append(t_dve)

        def emit_dequant(ot):
            # W[o_p, g, a] = sum_b alpha[o_p, g, b] * binary[o_p, g, a, b]
            # accumulated in bf16 (chunked by gh groups). Bit-plane 2 is
            # scaled on gpsimd in parallel with DVE doing planes 0/1.
            bt = b_tiles[ot]
            w_sb = wpool.tile([P, G, A], BF16, tag="w", name=f"w{ot}")
            for gh0 in range(0, G, gh):
                gsl = slice(gh0, gh0 + gh)
                t_sb = wpool.tile([P, gh, A], BF16, tag="t", name=f"t{ot}_{gh0}")
                t2_sb = wpool.tile([P, gh, A], BF16, tag="t2",
                                   name=f"t2_{ot}_{gh0}")

                def al_bc(b):
                    return al3[:, ot, gsl, b:b + 1].to_broadcast([P, gh, A])

                if pool_assist:
                    nc.gpsimd.tensor_tensor(t2_sb, bt[2][:, gsl], al_bc(2), mult)
                nc.vector.tensor_tensor(w_sb[:, gsl], bt[0][:, gsl], al_bc(0), mult)
                nc.vector.tensor_tensor(t_sb, bt[1][:, gsl], al_bc(1), mult)
                nc.vector.tensor_tensor(w_sb[:, gsl], w_sb[:, gsl], t_sb, add)
                if not pool_assist:
                    nc.vector.tensor_tensor(t2_sb, bt[2][:, gsl], al_bc(2), mult)
                nc.vector.tensor_tensor(w_sb[:, gsl], w_sb[:, gsl], t2_sb, add)
            return w_sb

        def emit_transpose(w_sb):
            # One bf16 PE transpose per group block into Wt[a_p, g, o]
            # (PSUM -> SBUF via ACT Identity copy).
            wt_sb = wtpool.tile([P, KT, P], BF16, tag="wt")
            for g in range(G):
                ps = ps_tr.tile([P, P], BF16)
                nc.tensor.matmul(ps, w_sb[:, g], identb, is_transpose=True,
                                 start=True, stop=True)
                nc.scalar.add(wt_sb[:, g, :], ps, 0.0)
            return wt_sb

        def emit_mm(wt_sb, ot, t):
            # out[o_p, t] = sum_k Wt[:, k, o]^T @ x[:, k, t]  (bf16, W-stationary)
            ps = ps_mm.tile([P, TS], F32)
            for k in range(KT):
                nc.tensor.matmul(ps, wt_sb[:, k, :], x_tiles[t][:, k, :],
                                 start=(k == 0), stop=(k == KT - 1))
            out_sb = opool.tile([P, TS], ODT)
            # PSUM -> SBUF with per-partition bias on the ACT engine.
            nc.scalar.add(out_sb, ps, bias_sb[:, ot:ot + 1])
            nc.gpsimd.dma_start(out_d[ot, :, t * TS:(t + 1) * TS], out_sb)

        # --- Cell schedule: emit (ot, t) cells in estimated-availability
        # order so the in-order PE stream matches the DMA/dequant feeds.
        if zigzag:
            cells = sorted(
                ((max(d_av[ot], x_av[t]), ot, t)
                 for ot in range(OT) for t in range(TB)),
                key=lambda c: (c[0], c[1], c[2]))
        else:
            cells = [(0.0, ot, t) for ot in range(OT) for t in range(TB)]

        # DVE stream: all dequants up-front, in order.
        w_tiles = [emit_dequant(ot) for ot in range(OT)]
        wt_tiles = [None] * OT
        for _, ot, t in cells:
            if wt_tiles[ot] is None:
                wt_tiles[ot] = emit_transpose(w_tiles[ot])
            emit_mm(wt_tiles[ot], ot, t)

    _legalize_waits(nc)
    return nc


def _stage_inputs(input, binary, alpha, bias, tb):
    TB, TS = tb, MS // tb
    bf16 = mybir.dt.np(BF16)
    fp8 = mybir.dt.np(FP8)
    x = np.ascontiguousarray(np.asarray(input, dtype=np.float32)).reshape(MS, I)
    # xt[t, p, k, s] = x[t*TS + s, k*128 + p]
    xt = np.ascontiguousarray(
        x.reshape(TB, TS, KT, P).transpose(0, 3, 2, 1)).astype(bf16)
    # binary is strictly +-1, exactly representable in fp8e4 — lossless cast.
    bperm = np.ascontiguousarray(
        np.asarray(binary, dtype=np.float32).transpose(0, 3, 1, 2)).astype(fp8)
    alpha = np.ascontiguousarray(np.asarray(alpha, dtype=np.float32))
    bias = np.asarray(bias, dtype=np.float32)

    in_maps = []
    for c in range(N_CORES):
        sl = slice(c * O_SH, (c + 1) * O_SH)
        in_maps.append({
            "xt": xt,
            "bperm": np.ascontiguousarray(bperm[sl]),
            "alpha": np.ascontiguousarray(alpha[sl]),
            "biassh": np.ascontiguousarray(bias[sl].reshape(OT, P)),
        })
    return in_maps


def kernel(input, binary, alpha, bias, _trace=False, **opts):
    key = tuple(sorted(opts.items()))
    if key not in _CACHED:
        _CACHED[key] = build_nc(**opts)
    nc = _CACHED[key]
    tb = opts.get("tb", 8)
    in_maps = _stage_inputs(input, binary, alpha, bias, tb)
    res = run_bass_kernel_spmd(nc, in_maps, core_ids=list(range(N_CORES)),
                               trace=_trace)
    # out core shard [OT, P, MS] -> [MS, O_SH]
    shards = [np.asarray(res.results[c]["out"], dtype=np.float32)
              .transpose(2, 0, 1).reshape(MS, O_SH)
              for c in range(N_CORES)]
    out = np.concatenate(shards, axis=1).reshape(B, S, O).astype(np.float32)
    kernel.last_result = res
    return out
